# revision 10
# baseline (speedup 1.0000x reference)
"""DAN classifier (embedding gather + mean-pool + tiny MLP + batch log-softmax)
on 8 Trainium2 NeuronCores.

Sharding: data-parallel over the batch (sentence) dim — 2048 sentences/core.
The embedding table is quantized to fp8 e4m3 (scaled by 16 to dodge the
subnormal band; the 1/16 is folded into V_w) and padded to [400000, 512] so
rows are 512 B — the dma_gather minimum 256 B-multiple covering 300 dims.
This is 0.4x the f32 baseline's gather bytes. MLP weights replicated.

Per-core device kernel (16 groups of 128 sentences, 6400 tokens each):
  - The vocab is split into 13 buckets of 32768 rows so row indices fit the
    int16 index format of InstDMAGatherAnt (which also caps num_idxs at
    1024 per instruction). The host buckets each group's tokens, pads each
    bucket list to a cross-core budget with index-0 dummies, and uploads
    int16 indices plus a bf16 per-slot sentence-id table.
  - 13 dma_gather ops per group (4 SWDGE queues round-robin) pull fp8 token
    rows into SBUF tiles [128, nblk, 512]: slot k -> partition k%128,
    block k//128.
  - Pooling: one-hot S in fp8 built on DVE (bf16 sentence ids vs iota).
    PE pools with fp8 DoubleRow matmuls (K=256 per instruction at 0.5
    cyc/col) over block pairs; trailing partial blocks run as regular
    matmuls with K=rem so unfetched slots are never read.
  - MLP: PE transpose of pooled -> [300, 128]; matmuls against
    V_w.T/(SEQ*16) (mean + fp8-scale fold), ReLU+bias on ACT, W matmul,
    W_b add on DVE.
  - One DMA writes logits.T [2, 2048] to DRAM.

Host glue: shard/pack tokens, run SPMD on cores 0-7, concatenate the logit
slabs and apply the global log-softmax over the batch axis.
"""

import numpy as np

VOCAB, DIM, HID, OUT = 400000, 300, 32, 2
BATCH, SEQ = 16384, 50
N_CORES = 8
B_CORE = BATCH // N_CORES            # 2048 sentences per core
GROUP = 128                          # sentences per group
N_GROUPS = B_CORE // GROUP           # 16
EPAD = 512                           # fp8 row bytes (== elements)
BUCKET = 32768                       # int16-addressable rows per bucket
NB = -(-VOCAB // BUCKET)             # 13
DCH = (128, 128, DIM - 256)          # contraction chunks over DIM
N_QUEUES = 4
EMB_SCALE = 16.0                     # fp8 pre-scale, folded out in V_w


def _cdiv(a, b):
    return -(-a // b)


QUAD = 4                             # groups merged per gather stream
CHUNK = 1024                         # dma_gather num_idxs cap
CBLK = CHUNK // 128                  # blocks per chunk (8)


def _quad_bucket_ops(nblks):
    """Walk one bucket's merged block stream (QUAD group sublists, each
    padded to whole blocks). Emit DR pairs within (group, chunk) runs,
    singles at run tails. Returns (gi, local_blk, chunk, pos, is_pair)."""
    ops = []
    j = 0
    for gi, nb in enumerate(nblks):
        lb = 0
        while lb < nb:
            c, pos = divmod(j, CBLK)
            if lb + 1 < nb and pos + 1 < CBLK:
                ops.append((gi, lb, c, pos, True))
                lb += 2
                j += 2
            else:
                ops.append((gi, lb, c, pos, False))
                lb += 1
                j += 1
    return ops


class _Plan:
    """Per-(group,bucket) budgets and packed-layout offsets shared by the
    host packer and the device builder."""

    def __init__(self, budgets):
        self.budgets = budgets                      # [n_groups][NB] ints
        ng = len(budgets)
        self.nblk = [[_cdiv(budgets[g][b], 128) for b in range(NB)]
                     for g in range(ng)]
        self.blk_off = []                           # sent blk offset per (g,b)
        self.nblk_g = []                            # blocks per group
        bo = 0
        for g in range(ng):
            row_b = []
            blk0 = bo
            for b in range(NB):
                row_b.append(bo)
                bo += self.nblk[g][b]
            self.blk_off.append(row_b)
            self.nblk_g.append(bo - blk0)
        self.nblk_tot = bo
        self.max_nblk_g = max(self.nblk_g)
        # quad-merged gather streams: per (quad, bucket) the 4 padded group
        # sublists concatenated, chunked at 1024 idxs (8 blocks)
        self.nquad = ng // QUAD
        self.qicol = []                             # idx col start per (q,b)
        self.qtblk = []                             # stream blocks per (q,b)
        io = 0
        for q in range(self.nquad):
            row_i, row_t = [], []
            for b in range(NB):
                t = sum(self.nblk[QUAD * q + i][b] for i in range(QUAD))
                row_i.append(io)
                row_t.append(t)
                io += t * 8                         # t*128 idxs / 16 rows
            self.qicol.append(row_i)
            self.qtblk.append(row_t)
        self.icols_tot = io
        # matmul count per group (start/stop bookkeeping)
        self.n_mm_g = [0] * ng
        for q in range(self.nquad):
            for b in range(NB):
                for gi, lb, c, pos, isp in _quad_bucket_ops(
                        [self.nblk[QUAD * q + i][b] for i in range(QUAD)]):
                    self.n_mm_g[QUAD * q + gi] += 1

    def key(self):
        return tuple(tuple(r) for r in self.budgets)


def _build_bass(plan, vocab=VOCAB, dim=DIM, hid=HID, nout=OUT,
                b_core=B_CORE, group=GROUP, n_cores=N_CORES):
    from contextlib import ExitStack

    import concourse.tile as tile
    from concourse import bacc, mybir

    f32 = mybir.dt.float32
    bf16 = mybir.dt.bfloat16
    fp8 = mybir.dt.float8e4
    i16 = mybir.dt.int16
    n_groups = b_core // group
    dch = DCH
    nch = len(dch)
    DR = mybir.MatmulPerfMode.DoubleRow

    nc = bacc.Bacc("TRN2", target_bir_lowering=False, debug=False,
                   enable_asserts=False, num_devices=n_cores,
                   num_swdge_queues=N_QUEUES)
    t_idx = nc.declare_dram_parameter("gidx", [128, plan.icols_tot], i16,
                                      isOutput=False)
    t_sent = nc.declare_dram_parameter("sent", [128, plan.nblk_tot], bf16,
                                       isOutput=False)
    t_iota = nc.declare_dram_parameter("iota", [128, group], bf16,
                                       isOutput=False)
    t_ident = nc.declare_dram_parameter("ident", [128, 128], f32,
                                        isOutput=False)
    t_emb = nc.declare_dram_parameter("embp", [vocab, EPAD], fp8,
                                      isOutput=False)
    t_vwt = nc.declare_dram_parameter("vwt", [128, nch * hid], f32,
                                      isOutput=False)
    t_vb = nc.declare_dram_parameter("vb", [hid, 1], f32, isOutput=False)
    t_wwt = nc.declare_dram_parameter("wwt", [hid, nout], f32, isOutput=False)
    t_wb = nc.declare_dram_parameter("wb", [nout, 1], f32, isOutput=False)
    t_out = nc.declare_dram_parameter("out", [nout, b_core], f32,
                                      isOutput=True)

    relu = mybir.ActivationFunctionType.Relu
    is_eq = mybir.AluOpType.is_equal

    with ExitStack() as ctx:
        tc = ctx.enter_context(tile.TileContext(nc))
        consts = ctx.enter_context(tc.tile_pool(name="consts", bufs=1))
        gpool = ctx.enter_context(tc.tile_pool(name="gather", bufs=20))
        spool = ctx.enter_context(tc.tile_pool(name="smat", bufs=2 * QUAD))
        sbp = ctx.enter_context(tc.tile_pool(name="sbwork", bufs=2))
        pp_pool = ctx.enter_context(tc.tile_pool(name="ppool", bufs=QUAD, space="PSUM"))
        pt_pool = ctx.enter_context(tc.tile_pool(name="ptpool", bufs=2, space="PSUM"))
        ph_pool = ctx.enter_context(tc.tile_pool(name="phpool", bufs=1, space="PSUM"))
        pl_pool = ctx.enter_context(tc.tile_pool(name="plpool", bufs=1, space="PSUM"))

        idx_sb = consts.tile([128, plan.icols_tot], i16)
        nc.sync.dma_start(idx_sb[:], t_idx[:])
        sent_sb = consts.tile([128, plan.nblk_tot], bf16)
        nc.sync.dma_start(sent_sb[:], t_sent[:])
        iota_sb = consts.tile([128, group], bf16)
        nc.sync.dma_start(iota_sb[:], t_iota[:])
        ident = consts.tile([128, 128], f32)
        nc.sync.dma_start(ident[:], t_ident[:])
        vwt_sb = consts.tile([128, nch * hid], f32)
        nc.sync.dma_start(vwt_sb[:], t_vwt[:])
        vb_sb = consts.tile([hid, 1], f32)
        nc.sync.dma_start(vb_sb[:], t_vb[:])
        wwt_sb = consts.tile([hid, nout], f32)
        nc.sync.dma_start(wwt_sb[:], t_wwt[:])
        wb_sb = consts.tile([nout, 1], f32)
        nc.sync.dma_start(wb_sb[:], t_wb[:])
        out_sb = consts.tile([nout, b_core], f32)

        # Compute instructions carry at most ONE embedded sync wait after
        # codegen. Prime each engine's vector clock on every external
        # producer it will consume mid-loop, so steady-state instructions
        # need only the wait on their data tile.
        dumb_dve = consts.tile([hid, 1], f32)
        nc.vector.tensor_copy(dumb_dve[0:nout, :], wb_sb[:])
        nc.vector.tensor_copy(dumb_dve[0:nout, :], sent_sb[0:nout, 0:1])
        nc.vector.tensor_copy(dumb_dve[0:nout, :], iota_sb[0:nout, 0:1])
        dumb_act = consts.tile([hid, 1], f32)
        nc.scalar.copy(dumb_act[:], vb_sb[:])
        dumb_ps = pl_pool.tile([nout, group], f32, tag="l")
        nc.tensor.matmul(dumb_ps[0:1, 0:1], lhsT=ident[:, 0:1],
                         rhs=ident[:, 0:1], start=True, stop=True)
        nc.tensor.matmul(dumb_ps[0:1, 0:1], lhsT=vwt_sb[:, 0:1],
                         rhs=vwt_sb[:, 0:1], start=True, stop=True)
        nc.tensor.matmul(dumb_ps[0:1, 0:1], lhsT=wwt_sb[:, 0:1],
                         rhs=wwt_sb[:, 0:1], start=True, stop=True)

        def build_s(g):
            """One-hot S for all blocks of group g: S[k, blk, s] =
            (sent[k, blk] == s), one DVE op, fp8 out."""
            nblk = plan.nblk_g[g]
            s_t = spool.tile([128, plan.max_nblk_g * group], fp8, tag="S")
            boff = plan.blk_off[g][0]
            in0 = sent_sb[:, boff:boff + nblk].to_broadcast([128, nblk, group])
            in1 = (iota_sb[:].rearrange("p (a c) -> p a c", a=1)
                   .to_broadcast([128, nblk, group]))
            nc.vector.tensor_tensor(
                out=s_t[:, 0:nblk * group].rearrange("p (c s) -> p c s",
                                                     s=group),
                in0=in0, in1=in1, op=is_eq)
            return s_t

        s_tiles = {g: build_s(g) for g in range(QUAD)}
        # prime PE on the DVE-built S
        nc.tensor.matmul(dumb_ps[0:1, 0:1], lhsT=s_tiles[0][:, 0:1],
                         rhs=s_tiles[0][:, 0:1], start=True, stop=True)

        gather_ct = 0
        for q in range(plan.nquad):
            chunk_tiles = {}
            for b in range(NB):
                t = plan.qtblk[q][b]
                io = plan.qicol[q][b]
                rows = min(BUCKET, vocab - b * BUCKET)
                for c in range(_cdiv(t, CBLK)):
                    kblk = min(CBLK, t - c * CBLK)
                    num = kblk * 128
                    gt = gpool.tile([128, CBLK * EPAD], fp8, tag="G")
                    nc.gpsimd.dma_gather(
                        out_ap=gt[:, 0:kblk * EPAD].rearrange(
                            "p (c e) -> p c e", e=EPAD),
                        in_ap=t_emb[b * BUCKET: b * BUCKET + rows, :],
                        idxs_ap=idx_sb[:, io + c * 64: io + c * 64 + num // 16],
                        num_idxs=num,
                        num_idxs_reg=num,
                        elem_size=EPAD,
                        queue_num=gather_ct % N_QUEUES,
                    )
                    gather_ct += 1
                    chunk_tiles[(b, c)] = gt

            # S for quad q+1 built now (DVE order: before this quad's
            # pooled/pt copies) so next quad's matmuls carry no DVE wait.
            if q + 1 < plan.nquad:
                for i in range(QUAD):
                    s_tiles[QUAD * (q + 1) + i] = build_s(QUAD * (q + 1) + i)

            pooled = {i: pp_pool.tile([group, dim], f32, tag="pooled",
                                      name=f"pooled_q{q}_{i}")
                      for i in range(QUAD)}
            mm = [0] * QUAD
            for b in range(NB):
                nblks = [plan.nblk[QUAD * q + i][b] for i in range(QUAD)]
                for gi, lb, c, pos, isp in _quad_bucket_ops(nblks):
                    g = QUAD * q + gi
                    s_g = s_tiles[g]
                    sblk0 = plan.blk_off[g][b] - plan.blk_off[g][0]
                    gt = chunk_tiles[(b, c)]
                    sc = sblk0 + lb
                    if isp:
                        nc.tensor.matmul(
                            pooled[gi][:],
                            lhsT=s_g[:, sc * group:(sc + 2) * group]
                                .rearrange("p (c s) -> p c s", s=group),
                            rhs=gt[:, pos * EPAD:(pos + 2) * EPAD]
                                .rearrange("p (c e) -> p c e", e=EPAD)
                                [:, :, 0:dim],
                            start=(mm[gi] == 0),
                            stop=(mm[gi] + 1 == plan.n_mm_g[g]),
                            perf_mode=DR,
                        )
                    else:
                        nc.tensor.matmul(
                            pooled[gi][:],
                            lhsT=s_g[:, sc * group:(sc + 1) * group],
                            rhs=gt[:, pos * EPAD: pos * EPAD + dim],
                            start=(mm[gi] == 0),
                            stop=(mm[gi] + 1 == plan.n_mm_g[g]),
                        )
                    mm[gi] += 1

            for gi in range(QUAD):
                g = QUAD * q + gi
                s_tiles.pop(g)
                pooled_ps = pooled[gi]
                pooled_sb = sbp.tile([group, dim], f32, tag="pooled_sb")
                nc.vector.tensor_copy(pooled_sb[:], pooled_ps[:])

                pt_ps = pt_pool.tile([128, nch * group], f32, tag="pt")
                for cc, w in enumerate(dch):
                    nc.tensor.transpose(
                        out=pt_ps[0:w, cc * group: (cc + 1) * group],
                        in_=pooled_sb[:, cc * 128: cc * 128 + w],
                        identity=ident[:group, :group],
                    )
                pt_sb = sbp.tile([128, nch * group], f32, tag="pt_sb")
                nc.vector.tensor_copy(pt_sb[:, 0:2 * group],
                                      pt_ps[:, 0:2 * group])
                nc.vector.tensor_copy(pt_sb[0:dch[2], 2 * group:3 * group],
                                      pt_ps[0:dch[2], 2 * group:3 * group])

                h_ps = ph_pool.tile([hid, group], f32, tag="h")
                for cc, w in enumerate(dch):
                    nc.tensor.matmul(
                        h_ps[:],
                        lhsT=vwt_sb[0:w, cc * hid: (cc + 1) * hid],
                        rhs=pt_sb[0:w, cc * group: (cc + 1) * group],
                        start=(cc == 0),
                        stop=(cc == nch - 1),
                    )
                h_sb = sbp.tile([hid, group], f32, tag="h_sb")
                nc.scalar.activation(h_sb[:], h_ps[:], relu,
                                     bias=vb_sb[:, 0:1])

                l_ps = pl_pool.tile([nout, group], f32, tag="l")
                nc.tensor.matmul(l_ps[:], lhsT=wwt_sb[:], rhs=h_sb[:],
                                 start=True, stop=True)
                nc.vector.tensor_tensor(
                    out=out_sb[:, g * group: (g + 1) * group],
                    in0=l_ps[:],
                    in1=wb_sb[:, 0:1].to_broadcast([nout, group]),
                    op=mybir.AluOpType.add,
                )

        nc.sync.dma_start(t_out[:], out_sb[:])
    nc.finalize()
    return nc


def _pack_weights(V_w, V_b, W_w, W_b, dim=DIM, hid=HID, nout=OUT, seq=SEQ):
    nch = len(DCH)
    vwt = (np.asarray(V_w, np.float32).T
           / np.float32(seq * EMB_SCALE)).astype(np.float32)
    vwt_packed = np.zeros((128, nch * hid), np.float32)
    off = 0
    for c, w in enumerate(DCH):
        vwt_packed[0:w, c * hid: (c + 1) * hid] = vwt[off: off + w]
        off += w
    wwt = np.ascontiguousarray(np.asarray(W_w, np.float32).T)
    vb = np.asarray(V_b, np.float32).reshape(hid, 1)
    wb = np.asarray(W_b, np.float32).reshape(nout, 1)
    return vwt_packed, vb, wwt, wb


def _plan_and_pack(tokens, b_core=B_CORE, group=GROUP, seq=SEQ):
    """Bucket every core's tokens; compute cross-core budgets; pack int16
    index and bf16 sentence-id tables per core."""
    import ml_dtypes

    n_cores = tokens.shape[0] // b_core
    n_groups = b_core // group
    toks = np.asarray(tokens, np.int64).reshape(n_cores, n_groups, group, seq)

    # per (core, group): stable-sort tokens by bucket
    flat = toks.reshape(n_cores, n_groups, group * seq)
    sent_of = np.broadcast_to(np.arange(group)[:, None],
                              (group, seq)).reshape(group * seq)
    buck = flat >> 15
    counts = np.zeros((n_cores, n_groups, NB), np.int64)
    for b in range(NB):
        counts[:, :, b] = (buck == b).sum(axis=2)
    budgets = counts.max(axis=0)                     # [n_groups, NB]
    plan = _Plan(budgets.tolist())

    gidx = np.zeros((n_cores, 128, plan.icols_tot), np.int16)
    sent = np.full((n_cores, 128, plan.nblk_tot), -1.0, np.float32)
    for c in range(n_cores):
        locs = [[None] * NB for _ in range(n_groups)]
        for g in range(n_groups):
            order = np.argsort(buck[c, g], kind="stable")
            stoks = flat[c, g][order]
            ssent = sent_of[order]
            pos = 0
            for b in range(NB):
                n = int(counts[c, g, b])
                nblk = plan.nblk[g][b]
                nslot = nblk * 128
                loc = np.zeros(nslot, np.int16)
                sen = np.full(nslot, -1.0, np.float32)
                loc[:n] = (stoks[pos:pos + n] & 32767).astype(np.int16)
                sen[:n] = ssent[pos:pos + n]
                pos += n
                locs[g][b] = loc
                bo = plan.blk_off[g][b]
                sent[c, :, bo:bo + nblk] = sen.reshape(nblk, 128).T
        # quad-merged idx streams, wrapped [i%16, io + i//16]
        for q in range(plan.nquad):
            for b in range(NB):
                w = np.concatenate(
                    [locs[QUAD * q + i][b] for i in range(QUAD)])
                cols = w.size // 16
                io = plan.qicol[q][b]
                gidx[c, :, io:io + cols] = np.tile(
                    w.reshape(cols, 16).T, (8, 1))
    return plan, gidx, sent.astype(ml_dtypes.bfloat16)


_STATE = {}


def kernel(tokens, emb, V_w, V_b, W_w, W_b, _trace=False):
    import ml_dtypes
    from concourse.bass_utils import run_bass_kernel_spmd

    tokens = np.asarray(tokens)
    emb = np.asarray(emb, np.float32)

    plan, gidx, sent = _plan_and_pack(tokens)
    vwt_packed, vb, wwt, wb = _pack_weights(V_w, V_b, W_w, W_b)

    embp = _STATE.get("embp")
    if embp is None or _STATE.get("embp_src") is not emb:
        embp = np.zeros((VOCAB, EPAD), ml_dtypes.float8_e4m3)
        embp[:, :DIM] = (emb * np.float32(EMB_SCALE)).astype(
            ml_dtypes.float8_e4m3)
        _STATE["embp"] = embp
        _STATE["embp_src"] = emb

    iota = np.ascontiguousarray(
        np.broadcast_to(np.arange(GROUP, dtype=np.float32), (128, GROUP))
    ).astype(ml_dtypes.bfloat16)
    ident = np.eye(128, dtype=np.float32)

    nc = None
    if _STATE.get("plan_key") == plan.key():
        nc = _STATE.get("nc")
    if nc is None:
        nc = _build_bass(plan)
        _STATE["nc"] = nc
        _STATE["plan_key"] = plan.key()

    in_maps = [
        {
            "gidx": np.ascontiguousarray(gidx[c]),
            "sent": np.ascontiguousarray(sent[c]),
            "iota": iota,
            "ident": ident,
            "embp": embp,
            "vwt": vwt_packed,
            "vb": vb,
            "wwt": wwt,
            "wb": wb,
        }
        for c in range(N_CORES)
    ]
    res = run_bass_kernel_spmd(nc, in_maps, core_ids=list(range(N_CORES)),
                               trace=_trace)
    _STATE["last_result"] = res

    logits = np.concatenate([r["out"].T for r in res.results], axis=0)

    # global log-softmax over the batch axis (LogSoftmax(dim=0))
    x = logits.astype(np.float64)
    m = x.max(axis=0, keepdims=True)
    lse = m + np.log(np.sum(np.exp(x - m), axis=0, keepdims=True))
    return (x - lse).astype(np.float32)
